# revision 6
# baseline (speedup 1.0000x reference)
"""NetVLAD forward kernel for 8 TRN2 NeuronCores (Bass/Tile).

Reference (per batch b of 32):
  s = x @ Wk + b         (1024, 64) logits;  softmax over k -> a
  v[d,k] = sum_n a[n,k] x[n,d] + (sum_n a[n,k]) * C[d,k]
  v /= ||v||_2 over d (per k);  out = flatten(v) / ||flatten(v)||_2

Sharding: data-parallel over batch B=32 across 8 cores (4 batches/core).
Wk, b, C are replicated. No collectives; host concatenates outputs.

Per-core pipeline (per batch):
  - load x_b as bf16 [128, 8, 512] (gpsimd casting DMA)
  - PE-transpose x chunks -> xT (bf16), mm1: sT[k, n] = Wk^T x^T  (PSUM f32)
  - ACT exp(sT + bias) -> eT bf16; PE-transpose eT -> e[n, k] chunks
  - DVE: Z = rowsum(e), a = e / Z (bf16)
  - mm2: vT[k, d] = sum_n a[n,k] x[n,d]  (PSUM f32, exact bf16-product accum)
    asum[k] = sum_n a[n,k] via ones-matmul
  - vT += asum * C^T; intra-norm over d (free dim); global norm via
    gss = sum_k S_k/(S_k+eps) computed with tiny PE matmuls
  - PE-transpose normalized vT -> v[d, k], DMA out
"""

import sys

sys.path.insert(0, "/opt/trn_rl_repo")

from contextlib import ExitStack

import numpy as np

import concourse.bass as bass
import concourse.bacc as bacc
import concourse.tile as tile
from concourse import mybir
from concourse.bass_utils import run_bass_kernel_spmd
from concourse.masks import make_identity

F32 = mybir.dt.float32
BF16 = mybir.dt.bfloat16
AX = mybir.AxisListType
OP = mybir.AluOpType
ACTF = mybir.ActivationFunctionType

B_PER_CORE = 4  # 32 batches / 8 cores
N = 1024  # H*W pixels per batch
D = 512
K = 64
EPS = 1e-12
N_CORES = 8
DEBUG_STAGE = 99  # bisection knob: 4=stop after mm2/asum, 5=after norm chain


def build_kernel():
    nc = bacc.Bacc()
    x = nc.declare_dram_parameter("x", [B_PER_CORE * N, D], F32, isOutput=False)
    wk = nc.declare_dram_parameter("wk", [D, K], F32, isOutput=False)
    bb = nc.declare_dram_parameter("bb", [K, 1], F32, isOutput=False)
    cc = nc.declare_dram_parameter("cc", [D, K], F32, isOutput=False)
    out = nc.declare_dram_parameter("out", [B_PER_CORE, D * K], F32, isOutput=True)

    with tile.TileContext(nc) as tc, ExitStack() as ctx:
        const = ctx.enter_context(tc.tile_pool(name="const", bufs=1))
        xpool = ctx.enter_context(tc.tile_pool(name="xpool", bufs=2))
        xts = ctx.enter_context(tc.tile_pool(name="xts", bufs=3))
        sbm = ctx.enter_context(tc.tile_pool(name="sbm", bufs=2))
        nrm = ctx.enter_context(tc.tile_pool(name="nrm", bufs=2))
        # PSUM pools: 8 banks total
        ps_xt = ctx.enter_context(tc.tile_pool(name="ps_xt", bufs=2, space="PSUM"))
        ps_e = ctx.enter_context(tc.tile_pool(name="ps_e", bufs=2, space="PSUM"))
        ps_64 = ctx.enter_context(tc.tile_pool(name="ps_64", bufs=2, space="PSUM"))
        ps_m = ctx.enter_context(tc.tile_pool(name="ps_m", bufs=2, space="PSUM"))

        # ---- constants ----
        id_bf = const.tile([128, 128], BF16)
        make_identity(nc, id_bf[:])
        id_f32 = const.tile([128, 128], F32)
        make_identity(nc, id_f32[:])

        wkb = const.tile([128, 4, K], BF16)  # Wk [d, k] d-chunked, bf16
        nc.gpsimd.dma_start(
            out=wkb[:], in_=wk[:].rearrange("(j p) k -> p j k", p=128)
        )
        b_sb = const.tile([K, 1], F32)
        nc.sync.dma_start(out=b_sb[:], in_=bb[:])

        ones_bf = const.tile([128, 1], BF16)
        nc.vector.memset(ones_bf[:], 1.0)
        ones64 = const.tile([K, 1], F32)
        nc.vector.memset(ones64[:], 1.0)
        ones_row = const.tile([1, K], F32)
        nc.vector.memset(ones_row[:], 1.0)
        eps_sb = const.tile([K, 1], F32)
        nc.vector.memset(eps_sb[:], float(EPS))

        # C^T [64, 512] f32 via 4 PE transposes
        c_nat = const.tile([128, 4, K], F32)
        nc.sync.dma_start(out=c_nat[:], in_=cc[:].rearrange("(j p) k -> p j k", p=128))
        ct_ps = ps_m.tile([K, D], F32, tag="misc")
        for j in range(4):
            nc.tensor.transpose(
                ct_ps[:, j * 128 : (j + 1) * 128], c_nat[:, j, :], id_f32[:]
            )
        ct_sb = const.tile([K, D], F32)
        nc.vector.tensor_copy(ct_sb[:], ct_ps[:])

        # ---- per-batch pipeline ----
        for b in range(B_PER_CORE):
            # load x_b as bf16 [128 p, 8 i, 512 d] (casting DMA on gpsimd)
            xb = xpool.tile([128, 8, D], BF16, tag="xb")
            nc.gpsimd.dma_start(
                out=xb[:],
                in_=x[b * N : (b + 1) * N, :].rearrange("(i p) d -> p i d", p=128),
            )

            a_sb = sbm.tile([128, 8, K], BF16, tag="a")
            z_all = sbm.tile([128, 8], F32, tag="z")
            invz = sbm.tile([128, 8], F32, tag="invz")

            for g in range(2):  # n-groups of 512 pixels
                # -- transposes + mm1: sT = Wk^T @ x^T --
                s_ps = ps_64.tile([K, 512], F32, tag="big64")
                for j in range(4):  # d-chunks
                    xt_ps = ps_xt.tile([128, 512], BF16, tag="xt")
                    for c in range(4):  # n-subtiles
                        nc.tensor.transpose(
                            xt_ps[:, c * 128 : (c + 1) * 128],
                            xb[:, g * 4 + c, j * 128 : (j + 1) * 128],
                            id_bf[:],
                        )
                    xt_sb = xts.tile([128, 512], BF16, tag="xt_sb")
                    if j % 2 == 0:
                        nc.scalar.copy(xt_sb[:], xt_ps[:])
                    else:
                        nc.vector.tensor_copy(xt_sb[:], xt_ps[:])
                    nc.tensor.matmul(
                        s_ps[:], wkb[:, j, :], xt_sb[:], start=(j == 0), stop=(j == 3)
                    )

                # -- exp(s + b) -> eT bf16 --
                eT = sbm.tile([K, 512], BF16, tag="eT")
                nc.scalar.activation(eT[:], s_ps[:], ACTF.Exp, bias=b_sb[:])

                # -- transpose eT -> e [128n, 64k] chunks; Z; a = e/Z --
                e_ps = ps_e.tile([128, 4, K], BF16, tag="e")
                for c in range(4):
                    nc.tensor.transpose(
                        e_ps[:, c, :], eT[:, c * 128 : (c + 1) * 128], id_bf[:64, :64]
                    )
                nc.vector.reduce_sum(
                    z_all[:, g * 4 : (g + 1) * 4], e_ps[:], axis=AX.X
                )
                nc.vector.reciprocal(
                    invz[:, g * 4 : (g + 1) * 4], z_all[:, g * 4 : (g + 1) * 4]
                )
                for c in range(4):
                    i = g * 4 + c
                    nc.vector.tensor_scalar_mul(
                        a_sb[:, i, :], e_ps[:, c, :], invz[:, i : i + 1]
                    )

            # -- mm2: vT[k, d] accumulate over 8 n-tiles; asum via ones --
            v_ps = ps_64.tile([K, D], F32, tag="big64")
            for i in range(8):
                nc.tensor.matmul(
                    v_ps[:], a_sb[:, i, :], xb[:, i, :], start=(i == 0), stop=(i == 7)
                )
            as_ps = ps_m.tile([K, 1], F32, tag="misc")
            for i in range(8):
                nc.tensor.matmul(
                    as_ps[:], a_sb[:, i, :], ones_bf[:], start=(i == 0), stop=(i == 7)
                )

            if DEBUG_STAGE == 4:
                dbg = nrm.tile([K, D], F32, tag="vf")
                nc.vector.tensor_copy(dbg[:], v_ps[:])
                nc.sync.dma_start(
                    out=out[b].rearrange("(k d) -> k d", k=K, d=D)[:, 0:D], in_=dbg[:]
                )
                continue
            # -- v = vT + asum * C^T ; intra + global norm --
            asum = nrm.tile([K, 1], F32, tag="s1")
            nc.vector.tensor_copy(asum[:], as_ps[:])
            vc = nrm.tile([K, D], F32, tag="vc")
            nc.vector.tensor_scalar_mul(vc[:], ct_sb[:], asum[:])
            v_sb = nrm.tile([K, D], F32, tag="v")
            nc.vector.tensor_add(v_sb[:], vc[:], v_ps[:])

            # S_k = sum_d v^2
            sq = nrm.tile([K, D], F32, tag="sq")
            s_k = nrm.tile([K, 1], F32, tag="s2")
            nc.vector.tensor_mul(sq[:], v_sb[:], v_sb[:])
            nc.vector.reduce_sum(s_k[:], sq[:], axis=AX.X)
            q = nrm.tile([K, 1], F32, tag="s3")
            nc.scalar.activation(q[:], s_k[:], ACTF.Sqrt, bias=eps_sb[:])
            rsq = nrm.tile([K, 1], F32, tag="s4")
            nc.vector.reciprocal(rsq[:], q[:])
            # t_k = S_k / (S_k + eps) = S_k * rsq^2
            t_k = nrm.tile([K, 1], F32, tag="s5")
            nc.vector.tensor_scalar(
                t_k[:], s_k[:], rsq[:], rsq[:], op0=OP.mult, op1=OP.mult
            )
            # gss = sum_k t_k  (tiny matmul), g = 1/sqrt(gss+eps), bcast to [64,1]
            gss_ps = ps_m.tile([1, 1], F32, tag="misc")
            nc.tensor.matmul(gss_ps[:], t_k[:], ones64[:], start=True, stop=True)
            gq = nrm.tile([1, 1], F32, tag="s6")
            nc.scalar.activation(gq[:], gss_ps[:], ACTF.Sqrt, bias=eps_sb[:1, :])
            ginv = nrm.tile([1, 1], F32, tag="s7")
            nc.vector.reciprocal(ginv[:], gq[:])
            gb_ps = ps_m.tile([K, 1], F32, tag="misc")
            nc.tensor.matmul(gb_ps[:], ones_row[:], ginv[:], start=True, stop=True)
            sc = nrm.tile([K, 1], F32, tag="s8")
            nc.vector.tensor_mul(sc[:], rsq[:], gb_ps[:])
            vf = nrm.tile([K, D], F32, tag="vf")
            nc.vector.tensor_scalar_mul(vf[:], v_sb[:], sc[:])
            if DEBUG_STAGE == 5:
                nc.sync.dma_start(
                    out=out[b].rearrange("(k d) -> k d", k=K, d=D)[:, 0:D], in_=vf[:]
                )
                continue

            # -- transpose back to [d, k] and store --
            o_sb = nrm.tile([128, 4, K], F32, tag="osb")
            for j in range(4):
                o_ps = ps_m.tile([128, K], F32, tag="misc")
                nc.tensor.transpose(
                    o_ps[:], vf[:, j * 128 : (j + 1) * 128], id_f32[:64, :64]
                )
                nc.scalar.copy(o_sb[:, j, :], o_ps[:])
            nc.sync.dma_start(
                out=out[b].rearrange("(j p k) -> p j k", j=4, p=128, k=K),
                in_=o_sb[:],
            )

    nc.compile()
    return nc


_CACHED_NC = None


def _get_nc():
    global _CACHED_NC
    if _CACHED_NC is None:
        _CACHED_NC = build_kernel()
    return _CACHED_NC


def kernel(x, Wk, b, C):
    """Full-input NetVLAD forward. x (32,32,32,512) f32 -> out (32, 32768) f32."""
    B = x.shape[0]
    x2 = np.ascontiguousarray(x, dtype=np.float32).reshape(B, N, D)
    bpc = B // N_CORES
    in_maps = []
    for c in range(N_CORES):
        in_maps.append(
            {
                "x": x2[c * bpc : (c + 1) * bpc].reshape(bpc * N, D),
                "wk": np.ascontiguousarray(Wk, dtype=np.float32),
                "bb": np.ascontiguousarray(b, dtype=np.float32).reshape(K, 1),
                "cc": np.ascontiguousarray(C, dtype=np.float32),
            }
        )
    nc = _get_nc()
    res = run_bass_kernel_spmd(nc, in_maps, list(range(N_CORES)))
    return np.concatenate([res.results[c]["out"] for c in range(N_CORES)], axis=0)
